# revision 1
# baseline (speedup 1.0000x reference)
"""MiniRocketFeatures Trainium2 kernel — data-parallel over batch on 8 NeuronCores.

Math per dilation d (pad = 4d, 26 dilations):
  C[b,k,t]   = sum_{c,j} W_d[(j,c), k] * x_pad[b, c, t + (j-4)*d]        (PE matmul, K=81)
  W_d[(j,c),k] = kernels[c*84+k, 0, j] * channel_combinations[d, c, k]   (host prep)
  count[b,k,f] = sum_t 1[C[b,k,t] > bias_d[k,f]]                         (DVE/ACT fused compare+row-sum)
  ppv = count / T   (T = 2048 for the "full" parity, 2048-8d for "valid") (host)

Batch-GROUP packing: one compare instruction covers up to three batches —
batch r of a group owns partitions 42r:42r+42 (SBUF APs must start at
0/32/64/96, so the instruction always spans [0:126) from partition 0; PSUM
matmul outputs are not quarter-constrained, so batch r's M=42 matmul writes
partitions 42r directly).  Per (group, dilation) the PE produces psF =
full-range conv of the full-parity kernels and psV = valid-range conv of the
valid-parity kernels; the group's first batch uses zero-padded M=126 weights
so every PSUM row is initialized, later batches overwrite their 42-row band.
ACT casts PSUM->SBUF bf16 (enables the DVE 4x compare mode) and also takes a
few compare instructions per group (Sign+accum, host converts (sum+T)/2) to
balance the engines.  Input DMAs alternate between the sync and gpsimd
queues so descriptor generation is not serialized on one sequencer.
"""

import numpy as np
import ml_dtypes

# ---- static MiniRocket config (matches reference.py; recomputed inline) ----
NUM_KERNELS = 84
KSIZE = 9
C_IN = 9
SEQ_LEN = 2048
BATCH = 64
N_CORES = 8
B_LOC = BATCH // N_CORES          # 8 batches per core
GROUPS = [(0, 1, 2), (3, 4, 5), (6, 7)]
PADW = 1024                        # zero padding each side of x (>= 4*max_d)
XPW = SEQ_LEN + 2 * PADW           # 4096
NCOL = 16                          # count columns reserved per dilation (>= max nf)
M = 126                            # partitions: 3 batches x 42 kernels


def _set_dilations(input_length):
    nfpk = 10000 // NUM_KERNELS
    tmd = min(nfpk, 32)
    multiplier = nfpk / tmd
    max_exponent = np.log2((input_length - 1) / (KSIZE - 1))
    dilations, counts = np.unique(
        np.logspace(0, max_exponent, tmd, base=2).astype(np.int32),
        return_counts=True)
    nfd = (counts * multiplier).astype(np.int32)
    remainder = nfpk - nfd.sum()
    i = 0
    while remainder > 0:
        nfd[i] += 1
        remainder -= 1
        i = (i + 1) % len(nfd)
    return [int(d) for d in dilations], [int(n) for n in nfd]


DILATIONS, NFD = _set_dilations(SEQ_LEN)
NUM_DIL = len(DILATIONS)           # 26
PADDINGS = [(KSIZE - 1) * d // 2 for d in DILATIONS]
# per-dilation kernel permutation: first 42 = full-range parity, last 42 = valid
PERMS = [list(range(i % 2, NUM_KERNELS, 2)) + list(range(1 - i % 2, NUM_KERNELS, 2))
         for i in range(NUM_DIL)]
# full-range compare instructions stolen by the ACT engine, keyed by dilation
STEAL = {1: 4, 2: 4, 5: 3, 3: 2, 4: 2}
ACT_F = [sorted(range(nf - STEAL.get(d, 0), nf)) for d, nf in zip(DILATIONS, NFD)]

_CACHE = {}


def _build_program():
    """Build the SPMD Bass/Tile program for one core."""
    from contextlib import ExitStack
    import concourse.bass as bass
    import concourse.bacc as bacc
    import concourse.tile as tile
    from concourse import mybir

    bf16 = mybir.dt.bfloat16
    fp8 = mybir.dt.float8e4
    f32 = mybir.dt.float32
    GT = mybir.AluOpType.is_gt
    ADD = mybir.AluOpType.add
    SIGN = mybir.ActivationFunctionType.Sign

    nc = bacc.Bacc("TRN2", target_bir_lowering=False, debug=False)
    xp = nc.declare_dram_parameter("xp", [2, C_IN, XPW, 4], bf16, isOutput=False)
    # w[i, :, v, r, :]: [81, 126] zero-padded weights, parity v (F/V), the 42
    # real columns placed at offset 42r so batch r PSUM-accumulates its band
    w = nc.declare_dram_parameter("w", [NUM_DIL, 81, 2, 3, M], bf16, isOutput=False)
    # bia[i]: [126, 48] = [F bias(16) | V bias(16) | -F bias for ACT(16)]
    bia = nc.declare_dram_parameter("bia", [NUM_DIL, M, 3 * NCOL], f32, isOutput=False)
    # out[group, {F,V,A}, partition, dil*16+f]
    out = nc.declare_dram_parameter("out", [len(GROUPS), 3, M, NUM_DIL * NCOL], f32,
                                    isOutput=True)

    def permuted(ap, order):
        return bass.AP(tensor=ap.tensor, offset=ap.offset,
                       ap=[ap.ap[i] for i in order])

    with tile.TileContext(nc) as tc, ExitStack() as ctx:
        singles = ctx.enter_context(tc.tile_pool(name="singles", bufs=1))
        xpool = ctx.enter_context(tc.tile_pool(name="xshift", bufs=2))
        cpool = ctx.enter_context(tc.tile_pool(name="csb", bufs=3))
        spool = ctx.enter_context(tc.tile_pool(name="scratch", bufs=3))
        ppool = ctx.enter_context(tc.tile_pool(name="psum", bufs=1, space="PSUM"))

        w_sb = singles.tile([81, NUM_DIL, 2, 3, M], bf16)
        nc.sync.dma_start(out=w_sb[:], in_=permuted(w[:, :, :, :, :], [1, 0, 2, 3, 4]))
        b_sb = singles.tile([M, NUM_DIL, 3 * NCOL], f32)
        nc.sync.dma_start(out=b_sb[:], in_=permuted(bia[:, :, :], [1, 0, 2]))

        NG = len(GROUPS)
        cntF = [singles.tile([M, NUM_DIL * NCOL], f32, name=f"cF{p}", tag=f"cF{p}")
                for p in range(NG)]
        cntV = [singles.tile([M, NUM_DIL * NCOL], f32, name=f"cV{p}", tag=f"cV{p}")
                for p in range(NG)]
        cntA = [singles.tile([M, NUM_DIL * NCOL], f32, name=f"cA{p}", tag=f"cA{p}")
                for p in range(NG)]
        for p in range(NG):
            nc.gpsimd.memset(cntF[p][:], 0.0)
            nc.gpsimd.memset(cntV[p][:], 0.0)
            nc.gpsimd.memset(cntA[p][:], 0.0)

        # interleave DVE-heavy (small d) and light (large d) dilations
        order = []
        lo_i, hi_i = 0, NUM_DIL - 1
        while lo_i <= hi_i:
            order.append(lo_i)
            if hi_i != lo_i:
                order.append(hi_i)
            lo_i += 1
            hi_i -= 1
        for i in order:
            d, nf = DILATIONS[i], NFD[i]
            vw = SEQ_LEN - 8 * d
            col0 = i * NCOL
            actf = ACT_F[i]
            dvef = [f for f in range(nf) if f not in actf]
            # shifted input tile: xt[9j+c, h, t, u] = xpad[4h+u, c, 1024+t+(j-4)d]
            # 2 half-batch DMAs per dilation, rotated over the 3 DMA queues
            xt = xpool.tile([81, 2, SEQ_LEN, 4], bf16, tag="xt")
            lo = PADW - 4 * d
            for h in range(2):
                anchor = xp[h:h + 1, 0:1, lo:lo + 1, 0:1]
                src = bass.AP(tensor=anchor.tensor, offset=anchor.offset,
                              ap=[[d * 4, KSIZE], [XPW * 4, C_IN],
                                  [1, SEQ_LEN * 4]])
                nc.scalar.dma_start(out=xt[:, h, :, :], in_=src)
            for p, grp in enumerate(GROUPS):
                psF = ppool.tile([M, SEQ_LEN], f32, tag="psF")
                psV = ppool.tile([M, SEQ_LEN], f32, tag="psV")
                ng = len(grp)
                for m in range(SEQ_LEN // 512):
                    for r, s in enumerate(grp):
                        nc.tensor.matmul(
                            psF[:, m * 512:(m + 1) * 512],
                            w_sb[:, i, 0, r, :],
                            xt[:, s // 4, m * 512:(m + 1) * 512, s % 4],
                            start=(r == 0), stop=(r == ng - 1))
                for q0 in range(0, vw, 512):
                    cw = min(512, vw - q0)
                    for r, s in enumerate(grp):
                        nc.tensor.matmul(
                            psV[:, q0:q0 + cw],
                            w_sb[:, i, 1, r, :],
                            xt[:, s // 4, 4 * d + q0:4 * d + q0 + cw, s % 4],
                            start=(r == 0), stop=(r == ng - 1))
                csF = cpool.tile([M, SEQ_LEN], bf16, tag="csF")
                nc.scalar.copy(out=csF[:], in_=psF[:])
                csV = cpool.tile([M, SEQ_LEN], bf16, tag="csV")
                nc.scalar.copy(out=csV[:, 0:vw], in_=psV[:, 0:vw])

                scr = spool.tile([M, SEQ_LEN], bf16, tag="scr")
                scrA = spool.tile([M, SEQ_LEN], bf16, tag="scrA")
                for f in dvef:
                    nc.vector.tensor_scalar(
                        out=scr[:], in0=csF[:],
                        scalar1=b_sb[:, i, f:f + 1], scalar2=None,
                        op0=GT, op1=ADD,
                        accum_out=cntF[p][:, col0 + f:col0 + f + 1])
                for f in actf:
                    nc.scalar.activation(
                        scrA[:], csF[:], SIGN,
                        bias=b_sb[:, i, 2 * NCOL + f:2 * NCOL + f + 1],
                        accum_out=cntA[p][:, col0 + f:col0 + f + 1])
                for f in range(nf):
                    nc.vector.tensor_scalar(
                        out=scr[:, 0:vw], in0=csV[:, 0:vw],
                        scalar1=b_sb[:, i, NCOL + f:NCOL + f + 1], scalar2=None,
                        op0=GT, op1=ADD,
                        accum_out=cntV[p][:, col0 + f:col0 + f + 1])

        for p in range(NG):
            nc.sync.dma_start(out=out[p, 0, :, :], in_=cntF[p][:])
            nc.sync.dma_start(out=out[p, 1, :, :], in_=cntV[p][:])
            nc.sync.dma_start(out=out[p, 2, :, :], in_=cntA[p][:])
    nc.compile()
    return nc


def _host_prep(x, kernels, channel_combinations, biases):
    """Build per-core input maps."""
    bf = ml_dtypes.bfloat16
    B = x.shape[0]
    xpad = np.zeros((B, C_IN, XPW), np.float32)
    xpad[:, :, PADW:PADW + SEQ_LEN] = x
    xpad = xpad.astype(bf)

    ker = np.asarray(kernels, np.float32).reshape(C_IN, NUM_KERNELS, KSIZE)
    cc = np.asarray(channel_combinations, np.float32)       # [26, 9, 84]
    bias = np.asarray(biases, np.float32)                   # [26, 84, maxnf]
    w_all = np.zeros((NUM_DIL, 81, 2, 3, M), np.float32)
    b_all = np.zeros((NUM_DIL, M, 3 * NCOL), np.float32)
    for i in range(NUM_DIL):
        pm = PERMS[i]
        # W[(j*9+c), k'] = ker[c, perm[k'], j] * cc[i, c, perm[k']]
        wk = ker[:, pm, :] * cc[i][:, pm, None]             # [9c, 84k', 9j]
        wk = wk.transpose(2, 0, 1).reshape(81, NUM_KERNELS)
        for r in range(3):
            w_all[i, :, 0, r, 42 * r:42 * r + 42] = wk[:, 0:42]    # F kernels
            w_all[i, :, 1, r, 42 * r:42 * r + 42] = wk[:, 42:84]   # V kernels
        nf = NFD[i]
        bF = bias[i][pm[:42], :nf]
        bV = bias[i][pm[42:], :nf]
        for r0 in (0, 42, 84):                              # same bias for all rows
            b_all[i, r0:r0 + 42, 0:nf] = bF
            b_all[i, r0:r0 + 42, NCOL:NCOL + nf] = bV
            b_all[i, r0:r0 + 42, 2 * NCOL:2 * NCOL + nf] = -bF
    w_all = w_all.astype(bf)

    in_maps = []
    for c in range(max(1, B // B_LOC)):
        xs = xpad[c * B_LOC:(c + 1) * B_LOC]          # [8, 9, 4096]
        xs = xs.reshape(2, 4, C_IN, XPW).transpose(0, 2, 3, 1)
        in_maps.append({"xp": np.ascontiguousarray(xs),
                        "w": w_all, "bia": b_all})
    return in_maps


def _host_post(out_all):
    """out_all [8*NG, 3, 126, 26*16] counts -> features in reference order."""
    n_cores = out_all.shape[0] // len(GROUPS)
    NB = n_cores * B_LOC
    # rows -> batch mapping: core c, group p, row band r -> batch c*B_LOC+GROUPS[p][r]
    feats = []
    for i, (d, nf) in enumerate(zip(DILATIONS, NFD)):
        cols = slice(i * NCOL, i * NCOL + nf)

        def both(sec):
            o = np.empty((NB, 42, nf), np.float32)
            for c in range(n_cores):
                for p, grp in enumerate(GROUPS):
                    for r, b in enumerate(grp):
                        o[c * B_LOC + b] = out_all[c * len(GROUPS) + p, sec,
                                                   42 * r:42 * r + 42, cols]
            return o
        full = both(0)
        if ACT_F[i]:
            acts = both(2)
            full[:, :, ACT_F[i]] = (acts[:, :, ACT_F[i]] + SEQ_LEN) * 0.5
        valid = both(1)
        feats.append((full / SEQ_LEN).reshape(NB, -1))
        feats.append((valid / (SEQ_LEN - 8 * d)).reshape(NB, -1))
    return np.concatenate(feats, axis=1).astype(np.float32)


def kernel(x, kernels, channel_combinations, biases):
    from concourse.bass_utils import run_bass_kernel_spmd

    if "nc" not in _CACHE:
        _CACHE["nc"] = _build_program()
    nc = _CACHE["nc"]

    in_maps = _host_prep(np.asarray(x, np.float32), kernels,
                         channel_combinations, biases)
    res = run_bass_kernel_spmd(nc, in_maps, core_ids=list(range(N_CORES)))
    out_all = np.concatenate([np.asarray(res.results[c]["out"], np.float32)
                              for c in range(N_CORES)], axis=0)
    return _host_post(out_all)

